# revision 14
# baseline (speedup 1.0000x reference)
"""Single-head attention (B=16, S=1024, D=768) on 8 Trainium2 NeuronCores.

Sharding: data-parallel over batch — each core computes 2 full batches
(QKV projection, S = q@k^T, softmax, P@V, output projection) with all
weights replicated. No collectives.

Layout strategy (all matmul operands float32r — full PE rate at N>=256,
~tf32 accuracy, PE rounds raw fp32 bits internally so no rounding passes):
  - x is host-transposed to xT [d, t] so the d-contraction runs directly.
  - q, k are produced transposed ([d, t]); v token-major ([t, d]).
  - S is computed TRANSPOSED ([j, i] = keys on partitions) so exp(S) lands
    directly in the layout P@V needs — no on-chip transpose of P.
  - softmax denominator via a ones-vector matmul on the PE (column sums);
    the normalization is applied to the P@V output (outT), keeping the
    reciprocal/broadcast chain off the PE critical path.
  - scale 1/sqrt(D) is folded into w_q/b_q on the host; biases b_q/b_k are
    per-partition ACT bias during the PSUM->SBUF copy; b_v is folded into
    b_out on the host (b_out_eff = b_out + w_out @ b_v); b_out_eff enters
    via a rank-1 ones x b_out matmul accumulated into the PSUM.
"""

import sys

import numpy as np

if "/opt/trn_rl_repo" not in sys.path:
    sys.path.insert(0, "/opt/trn_rl_repo")

import concourse.bass as bass  # noqa: E402
import concourse.bass_isa as bass_isa  # noqa: E402
import concourse.mybir as mybir  # noqa: E402
import concourse.tile as tile  # noqa: E402
from concourse import bacc  # noqa: E402
from concourse.bass_interp import get_hw_module  # noqa: E402
from concourse.bass_utils import run_bass_kernel_spmd  # noqa: E402

N_CORES = 8
B, S, D = 16, 1024, 768
BL = B // N_CORES  # batches per core
E3 = 3 * D
KT = D // 128  # 6 contraction tiles
F32 = mybir.dt.float32
F32R = mybir.dt.float32r

_prog = None


def _build():
    nc = bacc.Bacc("TRN2", target_bir_lowering=False, debug=False, num_devices=N_CORES)
    # pre-tiled on host: every DMA below reads 128 contiguous per-partition
    # blocks (minimal descriptor count, full DMA bandwidth)
    xT_d = nc.dram_tensor("xTt", [BL, 2, 2, 128, 3, 512], F32R,
                          kind="ExternalInput").ap()
    wqk_d = nc.dram_tensor("wqkt", [6, 128, KT, 256], F32R,
                           kind="ExternalInput").ap()
    wv_d = nc.dram_tensor("wvt", [2, 128, KT, 384], F32R,
                          kind="ExternalInput").ap()
    wout_d = nc.dram_tensor("woutt", [128, KT, D], F32R, kind="ExternalInput").ap()
    bqkv_d = nc.dram_tensor("bqkv", [128, 12], F32, kind="ExternalInput").ap()
    bout_d = nc.dram_tensor("bout", [128, D], F32, kind="ExternalInput").ap()
    y_d = nc.dram_tensor("y", [BL, S, D], F32, kind="ExternalOutput").ap()

    Exp = mybir.ActivationFunctionType.Exp
    Ident = mybir.ActivationFunctionType.Identity
    Mult = mybir.AluOpType.mult

    with tile.TileContext(nc) as tc:
        with tc.tile_pool(name="consts", bufs=1) as consts, \
             tc.tile_pool(name="wqk", bufs=2) as wqkp, \
             tc.tile_pool(name="wv", bufs=2) as wvp, \
             tc.tile_pool(name="xT", bufs=2) as xp, \
             tc.tile_pool(name="qk", bufs=1) as qkp, \
             tc.tile_pool(name="v", bufs=1) as vp, \
             tc.tile_pool(name="pt", bufs=2) as ptp, \
             tc.tile_pool(name="ot", bufs=1) as otp, \
             tc.tile_pool(name="y", bufs=2) as yp, \
             tc.tile_pool(name="small", bufs=1) as smallp, \
             tc.tile_pool(name="mm", bufs=8, space="PSUM") as mmp:

            b_sb = consts.tile([128, 12], F32)
            bout_sb = consts.tile([128, D], F32)
            w_out_sb = consts.tile([128, KT, D], F32R)

            for b in range(BL):
                # first weight slice before the big x DMAs so the PE can
                # start as soon as x-half 0 lands
                wsl0 = wqkp.tile([128, KT, 256], F32R, tag="wqk")
                # two half-slice DMAs: the first 128 e-columns land sooner,
                # letting the very first matmul chain start earlier
                nc.sync.dma_start(wsl0[:, :, :128], wqk_d[0, :, :, :128])
                nc.sync.dma_start(wsl0[:, :, 128:], wqk_d[0, :, :, 128:])
                if b == 0:
                    nc.sync.dma_start(b_sb[:], bqkv_d[:])
                # x halves (t in [0,512) and [512,1024)), prefetchable;
                # on the gpsimd DMA path so the weight stream on sync is
                # never queued behind these big transfers. Each half is two
                # 3-kt chunks so the first matmuls can start sooner.
                xh = []
                for h in range(2):
                    t = xp.tile([128, KT, 512], F32R, tag="xT")
                    for kc in range(2):
                        nc.sync.dma_start(t[:, 3 * kc:3 * (kc + 1)],
                                          xT_d[b, h, kc])
                    xh.append(t)
                qT = qkp.tile([128, KT, S], F32R, tag="qT")
                kT = qkp.tile([128, KT, S], F32R, tag="kT")
                v_sb = vp.tile([128, 8, D], F32R, tag="v")

                # A-qk: qkvT[e, t] = w_qkv @ x^T for the q/k rows
                for ew in range(6):  # weight slices of 256 e-columns
                    if ew == 0:
                        wsl = wsl0
                    else:
                        wsl = wqkp.tile([128, KT, 256], F32R, tag="wqk")
                        nc.sync.dma_start(wsl[:], wqk_d[ew])
                    for eh in range(2):  # 128-wide halves of the slice
                        et = 2 * ew + eh
                        for th in range(2):
                            ps = mmp.tile([128, 512], F32, tag="mm")
                            for kt in range(KT):
                                nc.tensor.matmul(ps[:], wsl[:, kt, 128 * eh:128 * (eh + 1)],
                                                 xh[th][:, kt],
                                                 start=(kt == 0), stop=(kt == KT - 1))
                            dst = qT if et < 6 else kT
                            nc.scalar.activation(dst[:, et % 6, 512 * th:512 * (th + 1)],
                                                 ps[:], Ident, bias=b_sb[:, et:et + 1])

                if b == 0:
                    nc.sync.dma_start(w_out_sb[:], wout_d[:])
                    nc.sync.dma_start(bout_sb[:], bout_d[:])

                # A-v: v[t, dv] token-major
                for dv2, (doff, dsz) in enumerate(((0, 384), (384, 384))):
                    wv = wvp.tile([128, KT, 384], F32R, tag="wv")
                    nc.sync.dma_start(wv[:], wv_d[dv2])
                    for tt in range(8):
                        ps = mmp.tile([128, 512], F32, tag="mm")
                        for kt in range(KT):
                            nc.tensor.matmul(ps[:, :dsz],
                                             xh[tt // 4][:, kt, 128 * (tt % 4):128 * (tt % 4 + 1)],
                                             wv[:, kt],
                                             start=(kt == 0), stop=(kt == KT - 1))
                        nc.vector.tensor_copy(v_sb[:, tt, doff:doff + dsz], ps[:, :dsz])

                for ih in range(2):
                    # B: S^T[j, i] tiles -> exp -> PT (unnormalized)
                    PT = ptp.tile([128, 8, 512], F32R, tag="PT")
                    for jt in range(8):
                        ps = mmp.tile([128, 512], F32, tag="mm")
                        for dt in range(KT):
                            nc.tensor.matmul(ps[:], kT[:, dt, 128 * jt:128 * (jt + 1)],
                                             qT[:, dt, 512 * ih:512 * (ih + 1)],
                                             start=(dt == 0), stop=(dt == KT - 1))
                        nc.scalar.activation(PT[:, jt], ps[:], Exp)

                    # C: softmax denominator: DVE pairwise add tree over the
                    # 8 jt tiles, then a gpsimd cross-partition all-reduce
                    Add = mybir.AluOpType.add
                    tree = smallp.tile([128, 4, 512], F32, tag="tree")
                    for p in range(4):
                        nc.vector.tensor_tensor(tree[:, p], PT[:, 2 * p],
                                                PT[:, 2 * p + 1], Add)
                    nc.vector.tensor_tensor(tree[:, 0], tree[:, 0], tree[:, 1], Add)
                    nc.vector.tensor_tensor(tree[:, 2], tree[:, 2], tree[:, 3], Add)
                    nc.vector.tensor_tensor(tree[:, 1], tree[:, 0], tree[:, 2], Add)
                    nc.gpsimd.partition_all_reduce(tree[:, 3], tree[:, 1], 128,
                                                   bass_isa.ReduceOp.add)
                    rb = smallp.tile([128, 512], F32, tag="rb")
                    nc.vector.reciprocal(rb[:], tree[:, 3])

                    # D: outT[dv, i] = (v^T @ P^T) * (1/denom) — normalized on DVE
                    outT = otp.tile([128, KT, 512], F32R, tag="outT")
                    for dvt in range(KT):
                        ps = mmp.tile([128, 512], F32, tag="mm")
                        for jt in range(8):
                            nc.tensor.matmul(ps[:], v_sb[:, jt, 128 * dvt:128 * (dvt + 1)],
                                             PT[:, jt], start=(jt == 0), stop=(jt == 7))
                        nc.vector.tensor_tensor(outT[:, dvt], ps[:], rb[:], Mult)

                    # E: y[t, e] = outT^T @ w_out^T + b_out_eff (rank-1 matmul bias)
                    for tt4 in range(4):
                        tt = 4 * ih + tt4
                        yt = yp.tile([128, D], F32, tag="y")
                        for eoff, esz in ((0, 512), (512, 256)):
                            ps = mmp.tile([128, 512], F32, tag="mm")
                            for dvt in range(KT):
                                nc.tensor.matmul(ps[:, :esz],
                                                 outT[:, dvt, 128 * tt4:128 * (tt4 + 1)],
                                                 w_out_sb[:, dvt, eoff:eoff + esz],
                                                 start=(dvt == 0), stop=(dvt == KT - 1))
                            nc.vector.tensor_tensor(yt[:, eoff:eoff + esz], ps[:, :esz],
                                                    bout_sb[:, eoff:eoff + esz],
                                                    mybir.AluOpType.add)
                        nc.scalar.dma_start(y_d[b, 128 * tt:128 * (tt + 1), :], yt[:])

    nc.compile()
    nc.m = get_hw_module(nc.m)
    return nc


def _prepare_in_maps(x, w_qkv, b_qkv, w_out, b_out):
    x = np.asarray(x, dtype=np.float32)
    w_qkv = np.asarray(w_qkv, dtype=np.float32)
    b_qkv = np.asarray(b_qkv, dtype=np.float32)
    w_out = np.asarray(w_out, dtype=np.float32)
    b_out = np.asarray(b_out, dtype=np.float32)

    s = D ** -0.5
    wq = np.ascontiguousarray(w_qkv.T)  # [D, 3D]
    wq[:, :D] *= s
    bqk = b_qkv[:2 * D].copy()
    bqk[:D] *= s
    b_arr = np.ascontiguousarray(bqk.reshape(12, 128).T)  # [128, 12]
    # pre-tiled weights: [slice, partition, ko, cols] with contiguous cols
    wqk_t = np.ascontiguousarray(
        wq[:, :2 * D].reshape(KT, 128, 6, 256).transpose(2, 1, 0, 3))
    wv_t = np.ascontiguousarray(
        wq[:, 2 * D:].reshape(KT, 128, 2, 384).transpose(2, 1, 0, 3))
    wout_t = np.ascontiguousarray(w_out.T.reshape(KT, 128, D).transpose(1, 0, 2))
    b_out_eff = (b_out + w_out @ b_qkv[2 * D:]).astype(np.float32)
    bout_arr = np.ascontiguousarray(np.broadcast_to(b_out_eff[None, :], (128, D)))

    in_maps = []
    for c in range(N_CORES):
        xl = x[BL * c:BL * (c + 1)]
        xT = xl.transpose(0, 2, 1)  # [BL, D, S]
        # [BL, h, kc, p, k3, t] with contiguous [k3, t] per partition
        xT_t = np.ascontiguousarray(
            xT.reshape(BL, 2, 3, 128, 2, 512).transpose(0, 4, 1, 3, 2, 5))
        in_maps.append({
            "xTt": xT_t, "wqkt": wqk_t, "wvt": wv_t, "woutt": wout_t,
            "bqkv": b_arr, "bout": bout_arr,
        })
    return in_maps


def _get_prog():
    global _prog
    if _prog is None:
        _prog = _build()
    return _prog


def _run(in_maps, **kwargs):
    res = run_bass_kernel_spmd(_get_prog(), in_maps, list(range(N_CORES)), **kwargs)
    return res


def kernel(x, w_qkv, b_qkv, w_out, b_out):
    in_maps = _prepare_in_maps(x, w_qkv, b_qkv, w_out, b_out)
    res = _run(in_maps)
    y = np.concatenate([res.results[c]["y"] for c in range(N_CORES)], axis=0)
    return y.astype(np.float32)


# revision 15
# speedup vs baseline: 1.0176x; 1.0176x over previous
"""Single-head attention (B=16, S=1024, D=768) on 8 Trainium2 NeuronCores.

Sharding: data-parallel over batch — each core computes 2 full batches
(QKV projection, S = q@k^T, softmax, P@V, output projection) with all
weights replicated. No collectives.

Layout strategy (all matmul operands float32r — full PE rate at N>=256,
~tf32 accuracy, PE rounds raw fp32 bits internally so no rounding passes):
  - x is host-transposed to xT [d, t] so the d-contraction runs directly.
  - q, k are produced transposed ([d, t]); v token-major ([t, d]).
  - S is computed TRANSPOSED ([j, i] = keys on partitions) so exp(S) lands
    directly in the layout P@V needs — no on-chip transpose of P.
  - softmax denominator via a ones-vector matmul on the PE (column sums);
    the normalization is applied to the P@V output (outT), keeping the
    reciprocal/broadcast chain off the PE critical path.
  - scale 1/sqrt(D) is folded into w_q/b_q on the host; biases b_q/b_k are
    per-partition ACT bias during the PSUM->SBUF copy; b_v is folded into
    b_out on the host (b_out_eff = b_out + w_out @ b_v); b_out_eff enters
    via a rank-1 ones x b_out matmul accumulated into the PSUM.
"""

import sys

import numpy as np

if "/opt/trn_rl_repo" not in sys.path:
    sys.path.insert(0, "/opt/trn_rl_repo")

import concourse.bass as bass  # noqa: E402
import concourse.bass_isa as bass_isa  # noqa: E402
import concourse.mybir as mybir  # noqa: E402
import concourse.tile as tile  # noqa: E402
from concourse import bacc  # noqa: E402
from concourse.bass_interp import get_hw_module  # noqa: E402
from concourse.bass_utils import run_bass_kernel_spmd  # noqa: E402

N_CORES = 8
B, S, D = 16, 1024, 768
BL = B // N_CORES  # batches per core
E3 = 3 * D
KT = D // 128  # 6 contraction tiles
F32 = mybir.dt.float32
F32R = mybir.dt.float32r

_prog = None


def _build():
    nc = bacc.Bacc("TRN2", target_bir_lowering=False, debug=False, num_devices=N_CORES)
    # pre-tiled on host: every DMA below reads 128 contiguous per-partition
    # blocks (minimal descriptor count, full DMA bandwidth)
    xT_d = nc.dram_tensor("xTt", [BL, 2, 2, 128, 3, 512], F32R,
                          kind="ExternalInput").ap()
    wqk_d = nc.dram_tensor("wqkt", [6, 128, KT, 256], F32R,
                           kind="ExternalInput").ap()
    wv_d = nc.dram_tensor("wvt", [2, 128, KT, 384], F32R,
                          kind="ExternalInput").ap()
    wout_d = nc.dram_tensor("woutt", [128, KT, D], F32R, kind="ExternalInput").ap()
    bqkv_d = nc.dram_tensor("bqkv", [128, 12], F32, kind="ExternalInput").ap()
    bout_d = nc.dram_tensor("bout", [128, D], F32, kind="ExternalInput").ap()
    y_d = nc.dram_tensor("y", [BL, S, D], F32, kind="ExternalOutput").ap()

    Exp = mybir.ActivationFunctionType.Exp
    Ident = mybir.ActivationFunctionType.Identity
    Mult = mybir.AluOpType.mult

    with tile.TileContext(nc) as tc:
        with tc.tile_pool(name="consts", bufs=1) as consts, \
             tc.tile_pool(name="wqk", bufs=2) as wqkp, \
             tc.tile_pool(name="wv", bufs=2) as wvp, \
             tc.tile_pool(name="xT", bufs=2) as xp, \
             tc.tile_pool(name="qk", bufs=1) as qkp, \
             tc.tile_pool(name="v", bufs=1) as vp, \
             tc.tile_pool(name="pt", bufs=2) as ptp, \
             tc.tile_pool(name="ot", bufs=1) as otp, \
             tc.tile_pool(name="y", bufs=2) as yp, \
             tc.tile_pool(name="small", bufs=1) as smallp, \
             tc.tile_pool(name="mm", bufs=8, space="PSUM") as mmp:

            b_sb = consts.tile([128, 12], F32)
            bout_sb = consts.tile([128, D], F32)
            w_out_sb = consts.tile([128, KT, D], F32R)

            for b in range(BL):
                # first weight slice before the big x DMAs so the PE can
                # start as soon as x-half 0 lands
                wsl0 = wqkp.tile([128, KT, 256], F32R, tag="wqk")
                nc.sync.dma_start(wsl0[:], wqk_d[0])
                if b == 0:
                    nc.sync.dma_start(b_sb[:], bqkv_d[:])
                # x halves (t in [0,512) and [512,1024)), prefetchable;
                # on the gpsimd DMA path so the weight stream on sync is
                # never queued behind these big transfers. Each half is two
                # 3-kt chunks so the first matmuls can start sooner.
                xh = []
                for h in range(2):
                    t = xp.tile([128, KT, 512], F32R, tag="xT")
                    for kc in range(2):
                        nc.sync.dma_start(t[:, 3 * kc:3 * (kc + 1)],
                                          xT_d[b, h, kc])
                    xh.append(t)
                qT = qkp.tile([128, KT, S], F32R, tag="qT")
                kT = qkp.tile([128, KT, S], F32R, tag="kT")
                v_sb = vp.tile([128, 8, D], F32R, tag="v")

                # A-qk: qkvT[e, t] = w_qkv @ x^T for the q/k rows
                for ew in range(6):  # weight slices of 256 e-columns
                    if ew == 0:
                        wsl = wsl0
                    else:
                        wsl = wqkp.tile([128, KT, 256], F32R, tag="wqk")
                        nc.sync.dma_start(wsl[:], wqk_d[ew])
                    for eh in range(2):  # 128-wide halves of the slice
                        et = 2 * ew + eh
                        for th in range(2):
                            ps = mmp.tile([128, 512], F32, tag="mm")
                            for kt in range(KT):
                                nc.tensor.matmul(ps[:], wsl[:, kt, 128 * eh:128 * (eh + 1)],
                                                 xh[th][:, kt],
                                                 start=(kt == 0), stop=(kt == KT - 1))
                            dst = qT if et < 6 else kT
                            nc.scalar.activation(dst[:, et % 6, 512 * th:512 * (th + 1)],
                                                 ps[:], Ident, bias=b_sb[:, et:et + 1])

                if b == 0:
                    nc.sync.dma_start(w_out_sb[:], wout_d[:])
                    nc.sync.dma_start(bout_sb[:], bout_d[:])

                # A-v: v[t, dv] token-major
                for dv2, (doff, dsz) in enumerate(((0, 384), (384, 384))):
                    wv = wvp.tile([128, KT, 384], F32R, tag="wv")
                    nc.sync.dma_start(wv[:], wv_d[dv2])
                    for tt in range(8):
                        ps = mmp.tile([128, 512], F32, tag="mm")
                        for kt in range(KT):
                            nc.tensor.matmul(ps[:, :dsz],
                                             xh[tt // 4][:, kt, 128 * (tt % 4):128 * (tt % 4 + 1)],
                                             wv[:, kt],
                                             start=(kt == 0), stop=(kt == KT - 1))
                        nc.vector.tensor_copy(v_sb[:, tt, doff:doff + dsz], ps[:, :dsz])

                for ih in range(2):
                    # B: S^T[j, i] tiles -> exp -> PT (unnormalized)
                    PT = ptp.tile([128, 8, 512], F32R, tag="PT")
                    for jt in range(8):
                        ps = mmp.tile([128, 512], F32, tag="mm")
                        for dt in range(KT):
                            nc.tensor.matmul(ps[:], kT[:, dt, 128 * jt:128 * (jt + 1)],
                                             qT[:, dt, 512 * ih:512 * (ih + 1)],
                                             start=(dt == 0), stop=(dt == KT - 1))
                        nc.scalar.activation(PT[:, jt], ps[:], Exp)

                    # C: softmax denominator: DVE pairwise add tree over the
                    # 8 jt tiles, then a gpsimd cross-partition all-reduce
                    Add = mybir.AluOpType.add
                    tree = smallp.tile([128, 4, 512], F32, tag="tree")
                    for p in range(4):
                        nc.vector.tensor_tensor(tree[:, p], PT[:, 2 * p],
                                                PT[:, 2 * p + 1], Add)
                    nc.vector.tensor_tensor(tree[:, 0], tree[:, 0], tree[:, 1], Add)
                    nc.vector.tensor_tensor(tree[:, 2], tree[:, 2], tree[:, 3], Add)
                    nc.vector.tensor_tensor(tree[:, 1], tree[:, 0], tree[:, 2], Add)
                    nc.gpsimd.partition_all_reduce(tree[:, 3], tree[:, 1], 128,
                                                   bass_isa.ReduceOp.add)
                    rb = smallp.tile([128, 512], F32, tag="rb")
                    nc.vector.reciprocal(rb[:], tree[:, 3])

                    # D: outT[dv, i] = (v^T @ P^T) * (1/denom) — normalized on DVE
                    outT = otp.tile([128, KT, 512], F32R, tag="outT")
                    for dvt in range(KT):
                        ps = mmp.tile([128, 512], F32, tag="mm")
                        for jt in range(8):
                            nc.tensor.matmul(ps[:], v_sb[:, jt, 128 * dvt:128 * (dvt + 1)],
                                             PT[:, jt], start=(jt == 0), stop=(jt == 7))
                        nc.vector.tensor_tensor(outT[:, dvt], ps[:], rb[:], Mult)

                    # E: y[t, e] = outT^T @ w_out^T + b_out_eff (rank-1 matmul bias)
                    for tt4 in range(4):
                        tt = 4 * ih + tt4
                        yt = yp.tile([128, D], F32, tag="y")
                        for eoff, esz in ((0, 512), (512, 256)):
                            ps = mmp.tile([128, 512], F32, tag="mm")
                            for dvt in range(KT):
                                nc.tensor.matmul(ps[:, :esz],
                                                 outT[:, dvt, 128 * tt4:128 * (tt4 + 1)],
                                                 w_out_sb[:, dvt, eoff:eoff + esz],
                                                 start=(dvt == 0), stop=(dvt == KT - 1))
                            nc.vector.tensor_tensor(yt[:, eoff:eoff + esz], ps[:, :esz],
                                                    bout_sb[:, eoff:eoff + esz],
                                                    mybir.AluOpType.add)
                        nc.scalar.dma_start(y_d[b, 128 * tt:128 * (tt + 1), :], yt[:])

    nc.compile()
    nc.m = get_hw_module(nc.m)
    return nc


def _prepare_in_maps(x, w_qkv, b_qkv, w_out, b_out):
    x = np.asarray(x, dtype=np.float32)
    w_qkv = np.asarray(w_qkv, dtype=np.float32)
    b_qkv = np.asarray(b_qkv, dtype=np.float32)
    w_out = np.asarray(w_out, dtype=np.float32)
    b_out = np.asarray(b_out, dtype=np.float32)

    s = D ** -0.5
    wq = np.ascontiguousarray(w_qkv.T)  # [D, 3D]
    wq[:, :D] *= s
    bqk = b_qkv[:2 * D].copy()
    bqk[:D] *= s
    b_arr = np.ascontiguousarray(bqk.reshape(12, 128).T)  # [128, 12]
    # pre-tiled weights: [slice, partition, ko, cols] with contiguous cols
    wqk_t = np.ascontiguousarray(
        wq[:, :2 * D].reshape(KT, 128, 6, 256).transpose(2, 1, 0, 3))
    wv_t = np.ascontiguousarray(
        wq[:, 2 * D:].reshape(KT, 128, 2, 384).transpose(2, 1, 0, 3))
    wout_t = np.ascontiguousarray(w_out.T.reshape(KT, 128, D).transpose(1, 0, 2))
    b_out_eff = (b_out + w_out @ b_qkv[2 * D:]).astype(np.float32)
    bout_arr = np.ascontiguousarray(np.broadcast_to(b_out_eff[None, :], (128, D)))

    in_maps = []
    for c in range(N_CORES):
        xl = x[BL * c:BL * (c + 1)]
        xT = xl.transpose(0, 2, 1)  # [BL, D, S]
        # [BL, h, kc, p, k3, t] with contiguous [k3, t] per partition
        xT_t = np.ascontiguousarray(
            xT.reshape(BL, 2, 3, 128, 2, 512).transpose(0, 4, 1, 3, 2, 5))
        in_maps.append({
            "xTt": xT_t, "wqkt": wqk_t, "wvt": wv_t, "woutt": wout_t,
            "bqkv": b_arr, "bout": bout_arr,
        })
    return in_maps


def _get_prog():
    global _prog
    if _prog is None:
        _prog = _build()
    return _prog


def _run(in_maps, **kwargs):
    res = run_bass_kernel_spmd(_get_prog(), in_maps, list(range(N_CORES)), **kwargs)
    return res


def kernel(x, w_qkv, b_qkv, w_out, b_out):
    in_maps = _prepare_in_maps(x, w_qkv, b_qkv, w_out, b_out)
    res = _run(in_maps)
    y = np.concatenate([res.results[c]["y"] for c in range(N_CORES)], axis=0)
    return y.astype(np.float32)


# revision 16
# speedup vs baseline: 1.0222x; 1.0045x over previous
"""Single-head attention (B=16, S=1024, D=768) on 8 Trainium2 NeuronCores.

Sharding: data-parallel over batch — each core computes 2 full batches
(QKV projection, S = q@k^T, softmax, P@V, output projection) with all
weights replicated. No collectives.

Layout strategy (all matmul operands float32r — full PE rate at N>=256,
~tf32 accuracy, PE rounds raw fp32 bits internally so no rounding passes):
  - x is host-transposed to xT [d, t] so the d-contraction runs directly.
  - q, k are produced transposed ([d, t]); v token-major ([t, d]).
  - S is computed TRANSPOSED ([j, i] = keys on partitions) so exp(S) lands
    directly in the layout P@V needs — no on-chip transpose of P.
  - softmax denominator via a DVE pairwise add tree + gpsimd cross-partition
    all-reduce; the normalization is applied to the P@V output (outT),
    keeping the reciprocal chain off the PE critical path.
  - scale 1/sqrt(D) is folded into w_q/b_q on the host; biases b_q/b_k are
    per-partition ACT bias during the PSUM->SBUF copy; b_v is folded into
    b_out on the host (b_out_eff = b_out + w_out @ b_v); b_out_eff is added
    on DVE during the final PSUM->SBUF copy from a host-broadcast tile.
  - all large inputs are host pre-tiled so every DMA reads 128 contiguous
    per-partition blocks (minimal descriptor count, full DMA bandwidth).
"""

import sys

import numpy as np

if "/opt/trn_rl_repo" not in sys.path:
    sys.path.insert(0, "/opt/trn_rl_repo")

import concourse.bass_isa as bass_isa  # noqa: E402
import concourse.mybir as mybir  # noqa: E402
import concourse.tile as tile  # noqa: E402
from concourse import bacc  # noqa: E402
from concourse.bass_interp import get_hw_module  # noqa: E402
from concourse.bass_utils import run_bass_kernel_spmd  # noqa: E402

N_CORES = 8
B, S, D = 16, 1024, 768
BL = B // N_CORES  # batches per core
E3 = 3 * D
KT = D // 128  # 6 contraction tiles
F32 = mybir.dt.float32
F32R = mybir.dt.float32r

_prog = None


def _build():
    nc = bacc.Bacc("TRN2", target_bir_lowering=False, debug=False, num_devices=N_CORES)
    # pre-tiled on host: every DMA below reads 128 contiguous per-partition
    # blocks (minimal descriptor count, full DMA bandwidth)
    xT_d = nc.dram_tensor("xTt", [BL, 2, 2, 128, 3, 512], F32R,
                          kind="ExternalInput").ap()
    wqk_d = nc.dram_tensor("wqkt", [6, 128, KT, 256], F32R,
                           kind="ExternalInput").ap()
    wv_d = nc.dram_tensor("wvt", [2, 128, KT, 384], F32R,
                          kind="ExternalInput").ap()
    wout_d = nc.dram_tensor("woutt", [128, KT, D], F32R, kind="ExternalInput").ap()
    bqkv_d = nc.dram_tensor("bqkv", [128, 12], F32, kind="ExternalInput").ap()
    bout_d = nc.dram_tensor("bout", [128, D], F32, kind="ExternalInput").ap()
    y_d = nc.dram_tensor("y", [BL, S, D], F32, kind="ExternalOutput").ap()

    Exp = mybir.ActivationFunctionType.Exp
    Ident = mybir.ActivationFunctionType.Identity
    Mult = mybir.AluOpType.mult

    with tile.TileContext(nc) as tc:
        with tc.tile_pool(name="consts", bufs=1) as consts, \
             tc.tile_pool(name="wqk", bufs=2) as wqkp, \
             tc.tile_pool(name="wv", bufs=2) as wvp, \
             tc.tile_pool(name="xT", bufs=2) as xp, \
             tc.tile_pool(name="qk", bufs=1) as qkp, \
             tc.tile_pool(name="v", bufs=1) as vp, \
             tc.tile_pool(name="pt", bufs=2) as ptp, \
             tc.tile_pool(name="ot", bufs=1) as otp, \
             tc.tile_pool(name="y", bufs=2) as yp, \
             tc.tile_pool(name="small", bufs=1) as smallp, \
             tc.tile_pool(name="mm", bufs=8, space="PSUM") as mmp:

            b_sb = consts.tile([128, 12], F32)
            bout_sb = consts.tile([128, D], F32)
            w_out_sb = consts.tile([128, KT, D], F32R)

            for b in range(BL):
                # first weight slice before the big x DMAs so the PE can
                # start as soon as x-half 0 lands
                wsl0 = wqkp.tile([128, KT, 256], F32R, tag="wqk")
                nc.sync.dma_start(wsl0[:], wqk_d[0])
                if b == 0:
                    nc.sync.dma_start(b_sb[:], bqkv_d[:])
                # x halves (t in [0,512) and [512,1024)), prefetchable;
                # on the gpsimd DMA path so the weight stream on sync is
                # never queued behind these big transfers. Each half is two
                # 3-kt chunks so the first matmuls can start sooner.
                xh = []
                for h in range(2):
                    t = xp.tile([128, KT, 512], F32R, tag="xT")
                    for kc in range(2):
                        nc.sync.dma_start(t[:, 3 * kc:3 * (kc + 1)],
                                          xT_d[b, h, kc])
                    xh.append(t)
                qT = qkp.tile([128, KT, S], F32R, tag="qT")
                kT = qkp.tile([128, KT, S], F32R, tag="kT")
                v_sb = vp.tile([128, 8, D], F32R, tag="v")

                # A-qk: qkvT[e, t] = w_qkv @ x^T for the q/k rows
                for ew in range(6):  # weight slices of 256 e-columns
                    if ew == 0:
                        wsl = wsl0
                    else:
                        wsl = wqkp.tile([128, KT, 256], F32R, tag="wqk")
                        nc.sync.dma_start(wsl[:], wqk_d[ew])
                    for eh in range(2):  # 128-wide halves of the slice
                        et = 2 * ew + eh
                        for th in range(2):
                            ps = mmp.tile([128, 512], F32, tag="mm")
                            for kt in range(KT):
                                nc.tensor.matmul(ps[:], wsl[:, kt, 128 * eh:128 * (eh + 1)],
                                                 xh[th][:, kt],
                                                 start=(kt == 0), stop=(kt == KT - 1))
                            dst = qT if et < 6 else kT
                            nc.scalar.activation(dst[:, et % 6, 512 * th:512 * (th + 1)],
                                                 ps[:], Ident, bias=b_sb[:, et:et + 1])

                if b == 0:
                    nc.sync.dma_start(w_out_sb[:], wout_d[:])
                    nc.sync.dma_start(bout_sb[:], bout_d[:])

                # A-v: v[t, dv] token-major
                for dv2, (doff, dsz) in enumerate(((0, 384), (384, 384))):
                    wv = wvp.tile([128, KT, 384], F32R, tag="wv")
                    nc.sync.dma_start(wv[:], wv_d[dv2])
                    for tt in range(8):
                        ps = mmp.tile([128, 512], F32, tag="mm")
                        for kt in range(KT):
                            nc.tensor.matmul(ps[:, :dsz],
                                             xh[tt // 4][:, kt, 128 * (tt % 4):128 * (tt % 4 + 1)],
                                             wv[:, kt],
                                             start=(kt == 0), stop=(kt == KT - 1))
                        nc.vector.tensor_copy(v_sb[:, tt, doff:doff + dsz], ps[:, :dsz])

                for ih in range(2):
                    # B: S^T[j, i] tiles -> exp -> PT (unnormalized)
                    PT = ptp.tile([128, 8, 512], F32R, tag="PT")
                    for jt in range(8):
                        ps = mmp.tile([128, 512], F32, tag="mm")
                        for dt in range(KT):
                            nc.tensor.matmul(ps[:], kT[:, dt, 128 * jt:128 * (jt + 1)],
                                             qT[:, dt, 512 * ih:512 * (ih + 1)],
                                             start=(dt == 0), stop=(dt == KT - 1))
                        nc.scalar.activation(PT[:, jt], ps[:], Exp)

                    # C: softmax denominator: DVE pairwise add tree over the
                    # 8 jt tiles, then a gpsimd cross-partition all-reduce
                    Add = mybir.AluOpType.add
                    tree = smallp.tile([128, 4, 512], F32, tag="tree")
                    for p in range(4):
                        nc.vector.tensor_tensor(tree[:, p], PT[:, 2 * p],
                                                PT[:, 2 * p + 1], Add)
                    nc.vector.tensor_tensor(tree[:, 0], tree[:, 0], tree[:, 1], Add)
                    nc.vector.tensor_tensor(tree[:, 2], tree[:, 2], tree[:, 3], Add)
                    nc.vector.tensor_tensor(tree[:, 1], tree[:, 0], tree[:, 2], Add)
                    nc.gpsimd.partition_all_reduce(tree[:, 3], tree[:, 1], 128,
                                                   bass_isa.ReduceOp.add)
                    rb = smallp.tile([128, 512], F32, tag="rb")
                    nc.vector.reciprocal(rb[:], tree[:, 3])

                    # D: outT[dv, i] = (v^T @ P^T) * (1/denom) — normalized on DVE
                    outT = otp.tile([128, KT, 512], F32R, tag="outT")
                    for dvt in range(KT):
                        ps = mmp.tile([128, 512], F32, tag="mm")
                        for jt in range(8):
                            nc.tensor.matmul(ps[:], v_sb[:, jt, 128 * dvt:128 * (dvt + 1)],
                                             PT[:, jt], start=(jt == 0), stop=(jt == 7))
                        nc.vector.tensor_tensor(outT[:, dvt], ps[:], rb[:], Mult)

                    # E: y[t, e] = outT^T @ w_out^T + b_out_eff (rank-1 matmul bias)
                    for tt4 in range(4):
                        tt = 4 * ih + tt4
                        yt = yp.tile([128, D], F32, tag="y")
                        for eoff, esz in ((0, 512), (512, 256)):
                            ps = mmp.tile([128, 512], F32, tag="mm")
                            for dvt in range(KT):
                                nc.tensor.matmul(ps[:, :esz],
                                                 outT[:, dvt, 128 * tt4:128 * (tt4 + 1)],
                                                 w_out_sb[:, dvt, eoff:eoff + esz],
                                                 start=(dvt == 0), stop=(dvt == KT - 1))
                            nc.vector.tensor_tensor(yt[:, eoff:eoff + esz], ps[:, :esz],
                                                    bout_sb[:, eoff:eoff + esz],
                                                    mybir.AluOpType.add)
                        nc.scalar.dma_start(y_d[b, 128 * tt:128 * (tt + 1), :], yt[:])

    nc.compile()
    nc.m = get_hw_module(nc.m)
    return nc


def _prepare_in_maps(x, w_qkv, b_qkv, w_out, b_out):
    x = np.asarray(x, dtype=np.float32)
    w_qkv = np.asarray(w_qkv, dtype=np.float32)
    b_qkv = np.asarray(b_qkv, dtype=np.float32)
    w_out = np.asarray(w_out, dtype=np.float32)
    b_out = np.asarray(b_out, dtype=np.float32)

    s = D ** -0.5
    wq = np.ascontiguousarray(w_qkv.T)  # [D, 3D]
    wq[:, :D] *= s
    bqk = b_qkv[:2 * D].copy()
    bqk[:D] *= s
    b_arr = np.ascontiguousarray(bqk.reshape(12, 128).T)  # [128, 12]
    # pre-tiled weights: [slice, partition, ko, cols] with contiguous cols
    wqk_t = np.ascontiguousarray(
        wq[:, :2 * D].reshape(KT, 128, 6, 256).transpose(2, 1, 0, 3))
    wv_t = np.ascontiguousarray(
        wq[:, 2 * D:].reshape(KT, 128, 2, 384).transpose(2, 1, 0, 3))
    wout_t = np.ascontiguousarray(w_out.T.reshape(KT, 128, D).transpose(1, 0, 2))
    b_out_eff = (b_out + w_out @ b_qkv[2 * D:]).astype(np.float32)
    bout_arr = np.ascontiguousarray(np.broadcast_to(b_out_eff[None, :], (128, D)))

    in_maps = []
    for c in range(N_CORES):
        xl = x[BL * c:BL * (c + 1)]
        xT = xl.transpose(0, 2, 1)  # [BL, D, S]
        # [BL, h, kc, p, k3, t] with contiguous [k3, t] per partition
        xT_t = np.ascontiguousarray(
            xT.reshape(BL, 2, 3, 128, 2, 512).transpose(0, 4, 1, 3, 2, 5))
        in_maps.append({
            "xTt": xT_t, "wqkt": wqk_t, "wvt": wv_t, "woutt": wout_t,
            "bqkv": b_arr, "bout": bout_arr,
        })
    return in_maps


def _get_prog():
    global _prog
    if _prog is None:
        _prog = _build()
    return _prog


def _run(in_maps, **kwargs):
    res = run_bass_kernel_spmd(_get_prog(), in_maps, list(range(N_CORES)), **kwargs)
    return res


def kernel(x, w_qkv, b_qkv, w_out, b_out):
    in_maps = _prepare_in_maps(x, w_qkv, b_qkv, w_out, b_out)
    res = _run(in_maps)
    y = np.concatenate([res.results[c]["y"] for c in range(N_CORES)], axis=0)
    return y.astype(np.float32)


# revision 17
# speedup vs baseline: 1.2141x; 1.1877x over previous
"""Single-head attention (B=16, S=1024, D=768) on 8 Trainium2 NeuronCores.

Sharding: data-parallel over batch — each core computes 2 full batches with
all weights replicated. No collectives.

Layout strategy (all matmul operands float32r — full PE rate, ~tf32
accuracy, PE rounds raw fp32 bits internally so no rounding passes):
  - x is host-transposed to xT [d, t] so the d-contraction runs directly.
  - q, k are produced transposed ([d, t]).
  - the output projection is FOLDED into the value projection on the host
    (wf = w_out @ w_v): the kernel computes vw = x @ wf^T token-major, and
    y^T = P @ vw needs one matmul stage instead of two (P@v then @w_out^T)
    — 15% fewer FLOPs. y is produced transposed; the host transposes back.
  - S is computed TRANSPOSED ([j, i] = keys on partitions) so exp(S) lands
    directly in the layout the P-contraction needs — no transpose of P.
  - softmax denominator via a DVE pairwise add tree + gpsimd cross-partition
    all-reduce; normalization multiplies the final y^T tiles (DVE), keeping
    the reciprocal chain off the PE critical path.
  - scale 1/sqrt(D) is folded into w_q/b_q on the host; biases b_q/b_k are
    per-partition ACT bias during the PSUM->SBUF copy; b_v and b_out fold
    into b_out_eff = b_out + w_out @ b_v, applied per-partition (ACT) on
    the transposed output.
  - all large inputs are host pre-tiled so every DMA reads 128 contiguous
    per-partition blocks (minimal descriptor count, full DMA bandwidth).
"""

import sys

import numpy as np

if "/opt/trn_rl_repo" not in sys.path:
    sys.path.insert(0, "/opt/trn_rl_repo")

import concourse.bass_isa as bass_isa  # noqa: E402
import concourse.mybir as mybir  # noqa: E402
import concourse.tile as tile  # noqa: E402
from concourse import bacc  # noqa: E402
from concourse.bass_interp import get_hw_module  # noqa: E402
from concourse.bass_utils import run_bass_kernel_spmd  # noqa: E402

N_CORES = 8
B, S, D = 16, 1024, 768
BL = B // N_CORES  # batches per core
KT = D // 128  # 6 contraction tiles
F32 = mybir.dt.float32
F32R = mybir.dt.float32r

_prog = None


def _build():
    nc = bacc.Bacc("TRN2", target_bir_lowering=False, debug=False, num_devices=N_CORES)
    # pre-tiled on host: every DMA below reads 128 contiguous per-partition
    # blocks (minimal descriptor count, full DMA bandwidth)
    xT_d = nc.dram_tensor("xTt", [BL, 2, 2, 128, 3, 512], F32R,
                          kind="ExternalInput").ap()
    wqk_d = nc.dram_tensor("wqkt", [6, 128, KT, 256], F32R,
                           kind="ExternalInput").ap()
    wf_d = nc.dram_tensor("wft", [2, 128, KT, 384], F32R,
                          kind="ExternalInput").ap()
    bqkv_d = nc.dram_tensor("bqkv", [128, 12], F32, kind="ExternalInput").ap()
    boute_d = nc.dram_tensor("boute", [128, KT], F32, kind="ExternalInput").ap()
    y_d = nc.dram_tensor("y", [BL, D, S], F32, kind="ExternalOutput").ap()

    Exp = mybir.ActivationFunctionType.Exp
    Ident = mybir.ActivationFunctionType.Identity
    Mult = mybir.AluOpType.mult
    Add = mybir.AluOpType.add

    with tile.TileContext(nc) as tc:
        with tc.tile_pool(name="consts", bufs=1) as consts, \
             tc.tile_pool(name="wqk", bufs=3) as wqkp, \
             tc.tile_pool(name="wf", bufs=2) as wfp, \
             tc.tile_pool(name="xT", bufs=3) as xp, \
             tc.tile_pool(name="qk", bufs=1) as qkp, \
             tc.tile_pool(name="vw", bufs=1) as vwp, \
             tc.tile_pool(name="pt", bufs=2) as ptp, \
             tc.tile_pool(name="y", bufs=3) as yp, \
             tc.tile_pool(name="small", bufs=1) as smallp, \
             tc.tile_pool(name="mm", bufs=8, space="PSUM") as mmp:

            b_sb = consts.tile([128, 12], F32)
            boute_sb = consts.tile([128, KT], F32)

            for b in range(BL):
                # first weight slice before the big x DMAs so the PE can
                # start as soon as x-half 0 lands
                wsl0 = wqkp.tile([128, KT, 256], F32R, tag="wqk")
                nc.sync.dma_start(wsl0[:], wqk_d[0])
                if b == 0:
                    nc.sync.dma_start(b_sb[:], bqkv_d[:])
                    nc.sync.dma_start(boute_sb[:], boute_d[:])
                # x halves (t in [0,512) and [512,1024)), prefetchable,
                # each as two 3-kt chunks so the first matmuls start sooner
                xh = []
                for h in range(2):
                    t = xp.tile([128, KT, 512], F32R, tag="xT")
                    for kc in range(2):
                        nc.sync.dma_start(t[:, 3 * kc:3 * (kc + 1)],
                                          xT_d[b, h, kc])
                    xh.append(t)
                qT = qkp.tile([128, KT, S], F32R, tag="qT")
                kT = qkp.tile([128, KT, S], F32R, tag="kT")
                vw_sb = vwp.tile([128, 8, D], F32R, tag="vw")

                # A-qk: qkvT[e, t] = w_qkv @ x^T for the q/k rows
                for ew in range(6):  # weight slices of 256 e-columns
                    if ew == 0:
                        wsl = wsl0
                    else:
                        wsl = wqkp.tile([128, KT, 256], F32R, tag="wqk")
                        nc.sync.dma_start(wsl[:], wqk_d[ew])
                    for eh in range(2):  # 128-wide halves of the slice
                        et = 2 * ew + eh
                        for th in range(2):
                            ps = mmp.tile([128, 512], F32, tag="mm")
                            for kt in range(KT):
                                nc.tensor.matmul(ps[:], wsl[:, kt, 128 * eh:128 * (eh + 1)],
                                                 xh[th][:, kt],
                                                 start=(kt == 0), stop=(kt == KT - 1))
                            dst = qT if et < 6 else kT
                            nc.scalar.activation(dst[:, et % 6, 512 * th:512 * (th + 1)],
                                                 ps[:], Ident, bias=b_sb[:, et:et + 1])

                # A-vw: vw[t, e] = x @ (w_out @ w_v)^T token-major
                for f2, (foff, fsz) in enumerate(((0, 384), (384, 384))):
                    wf = wfp.tile([128, KT, 384], F32R, tag="wf")
                    nc.sync.dma_start(wf[:], wf_d[f2])
                    for tt in range(8):
                        ps = mmp.tile([128, 512], F32, tag="mm")
                        for kt in range(KT):
                            nc.tensor.matmul(ps[:, :fsz],
                                             xh[tt // 4][:, kt, 128 * (tt % 4):128 * (tt % 4 + 1)],
                                             wf[:, kt],
                                             start=(kt == 0), stop=(kt == KT - 1))
                        nc.vector.tensor_copy(vw_sb[:, tt, foff:foff + fsz], ps[:, :fsz])

                for ih in range(2):
                    # B: S^T[j, i] tiles -> exp -> PT (unnormalized)
                    PT = ptp.tile([128, 8, 512], F32R, tag="PT")
                    for jt in range(8):
                        ps = mmp.tile([128, 512], F32, tag="mm")
                        for dt in range(KT):
                            nc.tensor.matmul(ps[:], kT[:, dt, 128 * jt:128 * (jt + 1)],
                                             qT[:, dt, 512 * ih:512 * (ih + 1)],
                                             start=(dt == 0), stop=(dt == KT - 1))
                        nc.scalar.activation(PT[:, jt], ps[:], Exp)

                    # C: softmax denominator: DVE pairwise add tree over the
                    # 8 jt tiles, then a gpsimd cross-partition all-reduce
                    tree = smallp.tile([128, 4, 512], F32, tag="tree")
                    for p in range(4):
                        nc.vector.tensor_tensor(tree[:, p], PT[:, 2 * p],
                                                PT[:, 2 * p + 1], Add)
                    nc.vector.tensor_tensor(tree[:, 0], tree[:, 0], tree[:, 1], Add)
                    nc.vector.tensor_tensor(tree[:, 2], tree[:, 2], tree[:, 3], Add)
                    nc.vector.tensor_tensor(tree[:, 1], tree[:, 0], tree[:, 2], Add)
                    nc.gpsimd.partition_all_reduce(tree[:, 3], tree[:, 1], 128,
                                                   bass_isa.ReduceOp.add)
                    rb = smallp.tile([128, 512], F32, tag="rb")
                    nc.vector.reciprocal(rb[:], tree[:, 3])

                    # D: y^T[e, i] = (vw^T @ P^T) * (1/denom) + b_out_eff
                    for et in range(KT):
                        ps = mmp.tile([128, 512], F32, tag="mm")
                        for jt in range(8):
                            nc.tensor.matmul(ps[:], vw_sb[:, jt, 128 * et:128 * (et + 1)],
                                             PT[:, jt], start=(jt == 0), stop=(jt == 7))
                        yt = yp.tile([128, 512], F32, tag="y")
                        nc.vector.tensor_tensor(yt[:], ps[:], rb[:], Mult)
                        nc.scalar.activation(yt[:], yt[:], Ident,
                                             bias=boute_sb[:, et:et + 1])
                        nc.scalar.dma_start(
                            y_d[b, 128 * et:128 * (et + 1), 512 * ih:512 * (ih + 1)],
                            yt[:])

    nc.compile()
    nc.m = get_hw_module(nc.m)
    return nc


def _prepare_in_maps(x, w_qkv, b_qkv, w_out, b_out):
    x = np.asarray(x, dtype=np.float32)
    w_qkv = np.asarray(w_qkv, dtype=np.float32)
    b_qkv = np.asarray(b_qkv, dtype=np.float32)
    w_out = np.asarray(w_out, dtype=np.float32)
    b_out = np.asarray(b_out, dtype=np.float32)

    s = D ** -0.5
    wq = np.ascontiguousarray(w_qkv.T)  # [D, 3D]
    wq[:, :D] *= s
    bqk = b_qkv[:2 * D].copy()
    bqk[:D] *= s
    b_arr = np.ascontiguousarray(bqk.reshape(12, 128).T)  # [128, 12]
    # folded value/output projection: wf[e, d] = (w_out @ w_v)[e, d]
    wf = w_out @ w_qkv[2 * D:, :]  # [D, D]
    wfT = np.ascontiguousarray(wf.T)  # [d, e]
    b_out_eff = (b_out + w_out @ b_qkv[2 * D:]).astype(np.float32)
    boute_arr = np.ascontiguousarray(b_out_eff.reshape(KT, 128).T)  # [128, KT]
    # pre-tiled weights: [slice, partition, ko, cols] with contiguous cols
    wqk_t = np.ascontiguousarray(
        wq[:, :2 * D].reshape(KT, 128, 6, 256).transpose(2, 1, 0, 3))
    wf_t = np.ascontiguousarray(
        wfT.reshape(KT, 128, 2, 384).transpose(2, 1, 0, 3))

    in_maps = []
    for c in range(N_CORES):
        xl = x[BL * c:BL * (c + 1)]
        xT = xl.transpose(0, 2, 1)  # [BL, D, S]
        # [BL, h, kc, p, k3, t] with contiguous [k3, t] per partition
        xT_t = np.ascontiguousarray(
            xT.reshape(BL, 2, 3, 128, 2, 512).transpose(0, 4, 1, 3, 2, 5))
        in_maps.append({
            "xTt": xT_t, "wqkt": wqk_t, "wft": wf_t,
            "bqkv": b_arr, "boute": boute_arr,
        })
    return in_maps


def _get_prog():
    global _prog
    if _prog is None:
        _prog = _build()
    return _prog


def _run(in_maps, **kwargs):
    res = run_bass_kernel_spmd(_get_prog(), in_maps, list(range(N_CORES)), **kwargs)
    return res


def kernel(x, w_qkv, b_qkv, w_out, b_out):
    in_maps = _prepare_in_maps(x, w_qkv, b_qkv, w_out, b_out)
    res = _run(in_maps)
    # kernel produces y transposed ([BL, D, S]); transpose back on host
    y = np.concatenate(
        [res.results[c]["y"].transpose(0, 2, 1) for c in range(N_CORES)], axis=0)
    return np.ascontiguousarray(y).astype(np.float32)
